# revision 3
# baseline (speedup 1.0000x reference)
"""Trainium2 Bass kernel for BilinearInteraction.

Computes out[b,p,:] = (x[:,pi[p],:] @ W[p]) * x[:,pj[p],:] for all P=276
field pairs (pi,pj) = combinations(24, 2), B=2048, E=128.

Strategy (8 NeuronCores):
  - Data-parallel: shard batch dim (2048 -> 256 rows per core), replicate W.
  - All inputs pre-arranged on host into SBUF-layout 2D arrays and cast to
    bf16 (free on host; halves DMA bytes).  W (9 MB bf16) stays resident in
    SBUF for the whole kernel.
  - Pairs sharing the same first field i are contiguous in p, so one
    stationary operand (x_i^T, [E=128 x 128 batch]) serves a whole group of
    matmuls whose moving operand is a contiguous slice of W.
  - Per 128-row batch chunk: matmul (bf16, fp32 PSUM) -> ScalarE copy with
    bf16 downcast into an SBUF staging buffer -> VectorE tensor-tensor
    multiply with x_j (bf16, 2x mode) -> single large DMA per ~70-pair
    stage to HBM (bf16; host upcasts to fp32).

The kernel is HBM-bandwidth bound (~30 MB/core total traffic); PE / ACT /
DVE work is hidden underneath the DMA.
"""

import numpy as np
import ml_dtypes

# ---------------------------------------------------------------- constants
F = 24          # fields
E = 128         # embedding dim
B = 2048        # batch
P = F * (F - 1) // 2        # 276 pairs
NCORES = 8
B_LOCAL = B // NCORES       # 256 rows per core
BCH = B_LOCAL // 128        # 2 batch chunks of 128
COLS = P * E                # 35328 output columns per batch chunk

# group g = pairs whose first field is g; sizes 23, 22, ..., 1
NGROUPS = F - 1
GS = [F - 1 - g for g in range(NGROUPS)]                  # pairs per group
GP = [0]
for s in GS:
    GP.append(GP[-1] + s)                                 # pair start per group

# stages: contiguous runs of whole groups, ~66-74 pairs each (=> ~2.3 MB
# bf16 output DMA per (batch chunk, stage))
STAGE_G = [(0, 3), (3, 7), (7, 12), (12, NGROUPS)]

PSUM_TILE = 2048            # fp32 elems per partition = 4 banks
BANK = 512                  # fp32 elems per PSUM bank
MM_MAX = 512                # max matmul free dim (fp32 PSUM bank)


def _build_schedule():
    """Static per-batch-chunk schedule.

    Returns list of stages; each stage is a dict with
      pair0, npairs, col0, cols,
      ptiles: list of (plcol0, pcols, segs) where segs is a list of
              (lcol0, n, g, start, stop)  [lcol relative to stage],
      groups: list of (g, glcol0, gcols).
    """
    stages = []
    for (glo, ghi) in STAGE_G:
        pair0 = GP[glo]
        npairs = GP[ghi] - GP[glo]
        col0 = pair0 * E
        cols = npairs * E

        # group boundaries local to this stage
        gb = [(GP[g] - pair0) * E for g in range(glo, ghi)] + [cols]
        groups = [
            (g, (GP[g] - pair0) * E, GS[g] * E) for g in range(glo, ghi)
        ]

        # psum tiles: local boundaries at multiples of PSUM_TILE
        ptiles = []
        pt0 = 0
        while pt0 < cols:
            pcols = min(PSUM_TILE, cols - pt0)
            # segments: split [pt0, pt0+pcols) at multiples of BANK (local to
            # stage, which psum-tile bases are aligned to) and at group bnds
            cuts = set()
            c = pt0
            while c < pt0 + pcols:
                cuts.add(c)
                c += BANK
            for b in gb:
                if pt0 < b < pt0 + pcols:
                    cuts.add(b)
            cuts = sorted(cuts) + [pt0 + pcols]
            segs = []
            for k in range(len(cuts) - 1):
                lcol0, n = cuts[k], cuts[k + 1] - cuts[k]
                # group containing this segment
                g = None
                for gi, gl0, gc in groups:
                    if gl0 <= lcol0 < gl0 + gc:
                        g = gi
                        break
                assert g is not None and n <= MM_MAX
                # span (PSUM bank) index relative to psum tile base
                span = (lcol0 - pt0) // BANK
                assert (lcol0 - pt0 + n - 1) // BANK == span
                segs.append([lcol0, n, g, span])
            # start/stop flags per span
            out_segs = []
            for k, (lcol0, n, g, span) in enumerate(segs):
                first = k == 0 or segs[k - 1][3] != span
                last = k == len(segs) - 1 or segs[k + 1][3] != span
                out_segs.append((lcol0, n, g, first, last))
            ptiles.append((pt0, pcols, out_segs))
            pt0 += pcols

        stages.append(
            dict(pair0=pair0, npairs=npairs, col0=col0, cols=cols,
                 ptiles=ptiles, groups=groups)
        )
    return stages


STAGES = _build_schedule()
MAX_STAGE_COLS = max(st["cols"] for st in STAGES)   # 9472

_NC = None


def _build_module():
    """Build (once) the Bass module: same program for all 8 cores."""
    global _NC
    if _NC is not None:
        return _NC

    import concourse.bass as bass
    import concourse.tile as tile
    from concourse import bacc, mybir

    bf = mybir.dt.bfloat16
    f32 = mybir.dt.float32

    nc = bacc.Bacc("TRN2", target_bir_lowering=False, debug=False)

    # host-prearranged SBUF-layout inputs (see kernel() for layouts)
    xT = nc.declare_dram_parameter("xT", [128, F * B_LOCAL], bf, isOutput=False)
    xn = nc.declare_dram_parameter("xn", [128, BCH * F * E], bf, isOutput=False)
    Wt = nc.declare_dram_parameter("Wt", [128, COLS], bf, isOutput=False)
    out = nc.declare_dram_parameter("out", [B_LOCAL, COLS], bf, isOutput=True)

    with tile.TileContext(nc) as tc:
        with (
            tc.tile_pool(name="const", bufs=1) as cpool,
            tc.tile_pool(name="mm", bufs=2) as mmpool,
            tc.tile_pool(name="so", bufs=2) as sopool,
            tc.tile_pool(name="ps", bufs=2, space=bass.MemorySpace.PSUM) as pspool,
        ):
            xT_sb = cpool.tile([128, F * B_LOCAL], bf, tag="xT")
            nc.sync.dma_start(out=xT_sb[:], in_=xT[:])
            xn_sb = cpool.tile([128, BCH * F * E], bf, tag="xn")
            nc.sync.dma_start(out=xn_sb[:], in_=xn[:])
            w_sb = []
            for s, st in enumerate(STAGES):
                w = cpool.tile([128, st["cols"]], bf, tag=f"w{s}")
                nc.sync.dma_start(
                    out=w[:], in_=Wt[:, st["col0"]: st["col0"] + st["cols"]]
                )
                w_sb.append(w)

            for bc in range(BCH):
                for s, st in enumerate(STAGES):
                    mm_t = mmpool.tile([128, st["cols"]], bf, tag="mm")
                    so_t = sopool.tile([128, st["cols"]], bf, tag="so")
                    for (plcol0, pcols, segs) in st["ptiles"]:
                        ps = pspool.tile([128, pcols], f32, tag="ps")
                        for (lcol0, n, g, first, last) in segs:
                            lhsT = xT_sb[
                                :, g * B_LOCAL + bc * 128: g * B_LOCAL + bc * 128 + 128
                            ]
                            rhs = w_sb[s][:, lcol0: lcol0 + n]
                            nc.tensor.matmul(
                                ps[:, lcol0 - plcol0: lcol0 - plcol0 + n],
                                lhsT,
                                rhs,
                                start=first,
                                stop=last,
                            )
                        nc.scalar.copy(
                            out=mm_t[:, plcol0: plcol0 + pcols], in_=ps[:]
                        )
                    for (g, glcol0, gcols) in st["groups"]:
                        nc.vector.tensor_mul(
                            so_t[:, glcol0: glcol0 + gcols],
                            mm_t[:, glcol0: glcol0 + gcols],
                            xn_sb[:, bc * F * E + (g + 1) * E:
                                  bc * F * E + (g + 1) * E + gcols],
                        )
                    nc.sync.dma_start(
                        out=out[bc * 128: (bc + 1) * 128,
                                st["col0"]: st["col0"] + st["cols"]],
                        in_=so_t[:],
                    )

    nc.compile()
    _NC = nc
    return nc


def _prep_inputs(x, W):
    """Host-side shard + relayout + bf16 cast. Returns in_maps for 8 cores."""
    bf = ml_dtypes.bfloat16
    x = np.ascontiguousarray(x, dtype=np.float32)
    W = np.ascontiguousarray(W, dtype=np.float32)

    # Wt[e, p*128+f] = W[p, e, f]
    Wt = np.ascontiguousarray(W.transpose(1, 0, 2).reshape(128, COLS)).astype(bf)

    in_maps = []
    for c in range(NCORES):
        xs = x[c * B_LOCAL: (c + 1) * B_LOCAL]            # [256, 24, 128]
        # xT[e, f*256+b] = xs[b, f, e]
        xT = np.ascontiguousarray(
            xs.transpose(2, 1, 0).reshape(128, F * B_LOCAL)
        ).astype(bf)
        # xn[b, bc*3072 + f*128 + e] = xs[bc*128+b, f, e]
        xn = np.ascontiguousarray(
            xs.reshape(BCH, 128, F, E).transpose(1, 0, 2, 3).reshape(128, BCH * F * E)
        ).astype(bf)
        in_maps.append({"xT": xT, "xn": xn, "Wt": Wt})
    return in_maps


def run_on_hw(x, W, trace=False, **run_kwargs):
    """Run the kernel on the 8 NeuronCores; returns (output fp32, results)."""
    from concourse.bass_utils import run_bass_kernel_spmd

    nc = _build_module()
    in_maps = _prep_inputs(x, W)
    res = run_bass_kernel_spmd(
        nc, in_maps, list(range(NCORES)), trace=trace, **run_kwargs
    )
    shards = []
    for c in range(NCORES):
        o = np.asarray(res.results[c]["out"])
        shards.append(o.astype(np.float32).reshape(B_LOCAL, P, E))
    return np.concatenate(shards, axis=0), res


def kernel(x, W):
    out, _ = run_on_hw(x, W, trace=False)
    return out
